# revision 9
# baseline (speedup 1.0000x reference)
"""Trainium2 Bass kernel: pixel-vs-memory-bank contrastive loss.

Math (equivalent to the reference, which builds the full [N,19,64] similarity
tensor):
  per-pixel loss  lp(n) = (1/64) * sum_m log(exp(pos_m) + sne) - negown
    pos_m   = f(n) . bank[k_n, m] / TEMP                    (own-class entries)
    sne     = sum_{j != k_n} exp(f(n) . mean_m bank[j] / TEMP)
    negown  = f(n) . mean_m bank[k_n] / TEMP  ( == mean_m pos_m )
  loss = mean_n lp(n)

So each pixel only needs a GEMM against 83 columns (64 own-class bank rows +
18 other-class bank means + 1 own-class mean) instead of all 19*64 = 1216 —
the loss is permutation-invariant over pixels, so the host groups pixels by
class, pads each class segment to a 128-pixel tile boundary, and splits the
work evenly across 8 cores with an identical static schedule on every core.
Padded (zero) pixels contribute exactly log(19) each and are subtracted on
the host.

Device layout per core:
  xp [128, 2, T*128]  f32 : xp[p, c2, n] = feat[channel c2*128+p, pixel n]
  wd [128, 2, 19*83]  f32 : per-class weight columns (pre-divided by TEMP)
  out llog [128, NU], nown [128, NU, 12] : partial sums, reduced on host.

Per 128-pixel tile of class k: out[px, j] = sum_c x[c,px] * w[c, 83k+j],
accumulated over the two 128-channel slices (2 matmuls into one PSUM slice).
Six tiles pack into one PSUM bank (6*83=498<=512); a unit = 2 banks = 12
tiles = 1536 pixels, processed with batched ACT/DVE ops:
  exp(neg18) -> reduce -> sne;  exp(pos64) + sne -> ln -> ACT-accum per unit.
"""

import os
import numpy as np

try:
    import concourse.bass as bass
except ImportError:  # fallback if PYTHONPATH lacks the repo
    import sys

    for _p in ("/opt/trn_rl_repo", "/root/.axon_site/_ro/trn_rl_repo"):
        if os.path.isdir(_p) and _p not in sys.path:
            sys.path.insert(0, _p)
    import concourse.bass as bass

import concourse.mybir as mybir
import concourse.tile as tile
from concourse.bass_utils import run_bass_kernel_spmd

TEMP = 100.0
B, C, H, W = 4, 256, 128, 128
K, M = 19, 64
NCORES = 8
P = 128
NPIX = B * H * W  # 65536
COLS = M + (K - 1) + 1  # 83 weight columns per class
TPB = 6  # tiles per PSUM bank (6*83 = 498 <= 512)
F32 = mybir.dt.float32

# bf16 feat/weights halve the HBM traffic and run the PE at 1 cycle/row
# (fp32 is 4); the quantization error washes out in the 65536-pixel mean
# (measured ~1e-7 relative on the final loss). KERNEL_FP32=1 to A/B.
if os.environ.get("KERNEL_FP32"):
    XDT = mybir.dt.float32
    _np_xdt = np.float32
else:
    import ml_dtypes

    XDT = mybir.dt.bfloat16
    _np_xdt = ml_dtypes.bfloat16

_prog_cache = {}


def _plan(mask_flat):
    """Class-grouped pixel layout with an identical schedule on all cores.

    Every class k gets cap_k = ceil(ceil(count_k/8)/128) tiles of 128 slots on
    every core; core c takes pixels idx_k[c::8]. Returns the per-class pixel
    lists, per-class slot offsets, the tile->class map, and the unit list
    (start_tile, n_banks, tiles_per_bank).
    """
    idx_by_class = [np.nonzero(mask_flat == k)[0] for k in range(K)]
    caps = [
        int(np.ceil(np.ceil(len(ix) / NCORES) / P)) if len(ix) else 0
        for ix in idx_by_class
    ]
    T = int(sum(caps))
    seg = np.concatenate([[0], np.cumsum(caps)]).astype(np.int64) * P
    tile_class = np.repeat(np.arange(K), caps)

    units = []
    t0 = 0
    while T - t0 >= 2 * TPB:
        units.append((t0, 2, TPB))
        t0 += 2 * TPB
    rem = T - t0
    if rem > TPB:
        units.append((t0, 1, TPB))
        t0 += TPB
        rem -= TPB
    if rem > 0:
        units.append((t0, 1, rem))
    return idx_by_class, caps, seg, tile_class, units, T


def _legalize_waits(nc):
    """Hoist extra sem-waits onto standalone EventSemaphore instructions.

    This walrus build accepts only ONE sync-wait per instruction
    ("Too many sync wait commands"); Tile emits 2-3 at phase boundaries.
    A same-engine EventSemaphore right before the instruction carries each
    extra wait — engines execute their block instructions in order, so the
    semantics are identical.
    """
    import bass_rust

    n = 0
    for f in nc.m.functions:
        for blk in f.blocks:
            insts = blk.instructions
            i = 0
            while i < len(insts):
                inst = insts[i]
                si = inst.sync_info
                if si is not None and len(si.on_wait) > 1:
                    waits = list(si.on_wait)
                    for w in waits[:-1]:
                        ev = mybir.InstEventSemaphore(
                            name=f"I-waitfix-{n}",
                            engine=inst.engine,
                            ins=[],
                            outs=[],
                            sync_info=bass_rust.SyncInfo(on_wait=[w], on_update=[]),
                        )
                        nc.register_instruction(ev, overwrite=True)
                        insts.insert(i, ev)
                        i += 1
                        n += 1
                    inst.sync_info = bass_rust.SyncInfo(
                        on_wait=[waits[-1]], on_update=list(si.on_update)
                    )
                i += 1
    return n


def _build(T, tile_class, units):
    """Emit the Bass/Tile program for one core (same program on all 8)."""
    NPX = T * P
    NU = len(units)
    nc = bass.Bass("TRN2", target_bir_lowering=False, debug=False)
    xp = nc.dram_tensor("xp", [P, 2, NPX], XDT, kind="ExternalInput").ap()
    wd = nc.dram_tensor("wd", [P, 2, K * COLS], XDT, kind="ExternalInput").ap()
    llog_d = nc.dram_tensor("llog", [P, NU], F32, kind="ExternalOutput").ap()
    nown_d = nc.dram_tensor("nown", [P, NU, 2 * TPB], F32, kind="ExternalOutput").ap()

    EXP = mybir.ActivationFunctionType.Exp
    LN = mybir.ActivationFunctionType.Ln

    with tile.TileContext(nc) as tc:
        with (
            tc.tile_pool(name="wpool", bufs=1) as wpool,
            # one slot per unit: loads never reuse a slot, so each DMA needs
            # no WAR/WAW wait (walrus allows only one sync-wait per DMA)
            tc.tile_pool(name="xpool", bufs=NU) as xpool,
            tc.tile_pool(name="ppool", bufs=3, space="PSUM") as ppool,
            tc.tile_pool(name="work", bufs=3) as work,
            tc.tile_pool(name="accs", bufs=1) as accs,
        ):
            wt = wpool.tile([P, 2, K * COLS], XDT)
            nc.sync.dma_start(wt[:], wd[:])
            llog_t = accs.tile([P, NU], F32)
            nown_t = accs.tile([P, NU, 2 * TPB], F32)

            for u, (t0, nb, tpb) in enumerate(units):
                g = nb * tpb
                ch = g * P
                xt = xpool.tile([P, 2, 2 * TPB * P], XDT, tag="xt")
                nc.gpsimd.dma_start(xt[:, :, 0:ch], xp[:, :, t0 * P : t0 * P + ch])
                ps = ppool.tile([P, 2, 512], F32, tag="ps")
                for t in range(g):
                    bk, ti = divmod(t, tpb)
                    kcls = int(tile_class[t0 + t])
                    c0 = ti * COLS
                    for c2 in range(2):
                        nc.tensor.matmul(
                            ps[:, bk, c0 : c0 + COLS],
                            xt[:, c2, t * P : (t + 1) * P],
                            wt[:, c2, kcls * COLS : (kcls + 1) * COLS],
                            start=(c2 == 0),
                            stop=(c2 == 1),
                        )
                psv = ps[:, 0:nb, 0 : tpb * COLS].rearrange(
                    "p b (t c) -> p b t c", c=COLS
                )
                neg = psv[:, :, :, M : M + K - 1]
                pos = psv[:, :, :, 0:M]
                own = psv[:, :, :, COLS - 1 : COLS]

                e = work.tile([P, 2, TPB, K - 1], F32, tag="e")
                ev = e[:, 0:nb, 0:tpb, :]
                nc.scalar.activation(ev, neg, EXP)
                sne = work.tile([P, 2, TPB], F32, tag="sne")
                snev = sne[:, 0:nb, 0:tpb]
                nc.vector.reduce_sum(snev, ev, axis=mybir.AxisListType.X)

                tb = work.tile([P, 2, TPB, M], F32, tag="tb")
                tbv = tb[:, 0:nb, 0:tpb, :]
                nc.scalar.activation(tbv, pos, EXP)
                nc.vector.tensor_add(
                    tbv, tbv, snev.unsqueeze(3).broadcast_to([P, nb, tpb, M])
                )
                nc.scalar.activation(tbv, tbv, LN, accum_out=llog_t[:, u : u + 1])

                nown_v = (
                    nown_t[:, u, 0:g].rearrange("p (b t) -> p b t", b=nb).unsqueeze(3)
                )
                nc.vector.tensor_copy(nown_v, own)

            nc.sync.dma_start(llog_d[:], llog_t[:])
            for u, (t0, nb, tpb) in enumerate(units):
                g = nb * tpb
                nc.sync.dma_start(nown_d[:, u, 0:g], nown_t[:, u, 0:g])
    _legalize_waits(nc)
    return nc


def prepare(feat, mask, bank):
    """Host-side: plan, per-core sharded inputs, weight matrix, pad count."""
    feat = np.ascontiguousarray(np.asarray(feat, dtype=np.float32))
    mask_flat = np.asarray(mask).reshape(-1).astype(np.int64)
    bank = np.asarray(bank, dtype=np.float32)

    idx_by_class, caps, seg, tile_class, units, T = _plan(mask_flat)
    NPX = T * P

    # [C, N] with the reference's pixel order n = (b*H + h)*W + w, then
    # [P, 2, N] so the DMA target layout matches SBUF directly.
    f3 = feat.transpose(1, 0, 2, 3).reshape(2, P, NPIX)
    xs = [np.zeros((P, 2, NPX), _np_xdt) for _ in range(NCORES)]
    for k in range(K):
        ix = idx_by_class[k]
        s = int(seg[k])
        for c in range(NCORES):
            ixc = ix[c::NCORES]
            xs[c][:, :, s : s + len(ixc)] = f3[:, :, ixc].transpose(1, 0, 2).astype(_np_xdt)
    n_pad_total = NCORES * NPX - NPIX

    bmean = bank.mean(axis=1)  # [K, C]
    wfull = np.zeros((C, K * COLS), np.float32)
    for k in range(K):
        wfull[:, k * COLS : k * COLS + M] = bank[k].T
        others = np.concatenate([np.arange(k), np.arange(k + 1, K)])
        wfull[:, k * COLS + M : k * COLS + M + K - 1] = bmean[others].T
        wfull[:, k * COLS + COLS - 1] = bmean[k]
    wfull /= TEMP
    wdat = np.ascontiguousarray(
        wfull.reshape(2, P, K * COLS).transpose(1, 0, 2).astype(_np_xdt)
    )

    return xs, wdat, tile_class, units, T, n_pad_total


def finish(results, n_pad_total, units):
    """Reduce per-core partial sums to the scalar loss (float64 on host)."""
    total = 0.0
    for r in results:
        total += r["llog"].sum(dtype=np.float64) / M
        for u, (t0, nb, tpb) in enumerate(units):
            total -= r["nown"][:, u, 0 : nb * tpb].sum(dtype=np.float64)
    total -= n_pad_total * np.log(19.0)
    return np.float32(total / NPIX)


def get_program(feat, mask, bank):
    xs, wdat, tile_class, units, T, n_pad_total = prepare(feat, mask, bank)
    key = (T, tuple(tile_class.tolist()))
    if key not in _prog_cache:
        _prog_cache[key] = _build(T, tile_class, units)
    return _prog_cache[key], xs, wdat, n_pad_total, units


def kernel(feat=None, mask=None, bank=None, _trace=False):
    nc, xs, wdat, n_pad_total, units = get_program(feat, mask, bank)
    in_maps = [{"xp": xs[c], "wd": wdat} for c in range(NCORES)]
    res = run_bass_kernel_spmd(
        nc, in_maps, core_ids=list(range(NCORES)), trace=_trace
    )
    loss = finish(res.results, n_pad_total, units)
    if _trace:
        return loss, res
    return loss


# revision 11
# speedup vs baseline: 1.0814x; 1.0814x over previous
"""Trainium2 Bass kernel: pixel-vs-memory-bank contrastive loss.

Math (equivalent to the reference, which builds the full [N,19,64] similarity
tensor):
  per-pixel loss  lp(n) = (1/64) * sum_m log(exp(pos_m) + sne) - negown
    pos_m   = f(n) . bank[k_n, m] / TEMP                    (own-class entries)
    sne     = sum_{j != k_n} exp(f(n) . mean_m bank[j] / TEMP)
    negown  = f(n) . mean_m bank[k_n] / TEMP  ( == mean_m pos_m )
  loss = mean_n lp(n)

So each pixel only needs a GEMM against 83 columns (64 own-class bank rows +
18 other-class bank means + 1 own-class mean) instead of all 19*64 = 1216 —
the loss is permutation-invariant over pixels, so the host groups pixels by
class, pads each class segment to a 128-pixel tile boundary, and splits the
work evenly across 8 cores with an identical static schedule on every core.
Padded (zero) pixels contribute exactly log(19) each and are subtracted on
the host.

Device layout per core:
  xp [128, 2, T*128]  f32 : xp[p, c2, n] = feat[channel c2*128+p, pixel n]
  wd [128, 2, 19*83]  f32 : per-class weight columns (pre-divided by TEMP)
  out llog [128, NU], nown [128, NU, 12] : partial sums, reduced on host.

Per 128-pixel tile of class k: out[px, j] = sum_c x[c,px] * w[c, 83k+j],
accumulated over the two 128-channel slices (2 matmuls into one PSUM slice).
Six tiles pack into one PSUM bank (6*83=498<=512); a unit = 2 banks = 12
tiles = 1536 pixels, processed with batched ACT/DVE ops:
  exp(neg18) -> reduce -> sne;  exp(pos64) + sne -> ln -> ACT-accum per unit.
"""

import os
import numpy as np

try:
    import concourse.bass as bass
except ImportError:  # fallback if PYTHONPATH lacks the repo
    import sys

    for _p in ("/opt/trn_rl_repo", "/root/.axon_site/_ro/trn_rl_repo"):
        if os.path.isdir(_p) and _p not in sys.path:
            sys.path.insert(0, _p)
    import concourse.bass as bass

import concourse.mybir as mybir
import concourse.tile as tile
from concourse.bass_utils import run_bass_kernel_spmd

TEMP = 100.0
B, C, H, W = 4, 256, 128, 128
K, M = 19, 64
NCORES = 8
P = 128
NPIX = B * H * W  # 65536
COLS = M + (K - 1) + 1  # 83 weight columns per class
TPB = 6  # tiles per PSUM bank (6*83 = 498 <= 512)
F32 = mybir.dt.float32

# bf16 feat/weights halve the HBM traffic and run the PE at 1 cycle/row
# (fp32 is 4); the quantization error washes out in the 65536-pixel mean
# (measured ~1e-7 relative on the final loss). KERNEL_FP32=1 to A/B.
if os.environ.get("KERNEL_FP32"):
    XDT = mybir.dt.float32
    _np_xdt = np.float32
else:
    import ml_dtypes

    XDT = mybir.dt.bfloat16
    _np_xdt = ml_dtypes.bfloat16

_prog_cache = {}


def _plan(mask_flat):
    """Class-grouped pixel layout with an identical schedule on all cores.

    Every class k gets cap_k = ceil(ceil(count_k/8)/128) tiles of 128 slots on
    every core; core c takes pixels idx_k[c::8]. Returns the per-class pixel
    lists, per-class slot offsets, the tile->class map, and the unit list
    (start_tile, n_banks, tiles_per_bank).
    """
    idx_by_class = [np.nonzero(mask_flat == k)[0] for k in range(K)]
    caps = [
        int(np.ceil(np.ceil(len(ix) / NCORES) / P)) if len(ix) else 0
        for ix in idx_by_class
    ]
    T = int(sum(caps))
    seg = np.concatenate([[0], np.cumsum(caps)]).astype(np.int64) * P
    tile_class = np.repeat(np.arange(K), caps)

    units = []
    t0 = 0
    while T - t0 >= 2 * TPB:
        units.append((t0, 2, TPB))
        t0 += 2 * TPB
    rem = T - t0
    if rem > TPB:
        units.append((t0, 1, TPB))
        t0 += TPB
        rem -= TPB
    if rem > 0:
        units.append((t0, 1, rem))
    return idx_by_class, caps, seg, tile_class, units, T


def _legalize_waits(nc):
    """Hoist extra sem-waits onto standalone EventSemaphore instructions.

    This walrus build accepts only ONE sync-wait per instruction
    ("Too many sync wait commands"); Tile emits 2-3 at phase boundaries.
    A same-engine EventSemaphore right before the instruction carries each
    extra wait — engines execute their block instructions in order, so the
    semantics are identical.
    """
    import bass_rust

    n = 0
    for f in nc.m.functions:
        for blk in f.blocks:
            insts = blk.instructions
            i = 0
            while i < len(insts):
                inst = insts[i]
                si = inst.sync_info
                if si is not None and len(si.on_wait) > 1:
                    waits = list(si.on_wait)
                    for w in waits[:-1]:
                        ev = mybir.InstEventSemaphore(
                            name=f"I-waitfix-{n}",
                            engine=inst.engine,
                            ins=[],
                            outs=[],
                            sync_info=bass_rust.SyncInfo(on_wait=[w], on_update=[]),
                        )
                        nc.register_instruction(ev, overwrite=True)
                        insts.insert(i, ev)
                        i += 1
                        n += 1
                    inst.sync_info = bass_rust.SyncInfo(
                        on_wait=[waits[-1]], on_update=list(si.on_update)
                    )
                i += 1
    return n


def _build(T, tile_class, units):
    """Emit the Bass/Tile program for one core (same program on all 8)."""
    NPX = T * P
    NU = len(units)
    nc = bass.Bass("TRN2", target_bir_lowering=False, debug=False)
    CHF = 2 * TPB * P
    xp = nc.dram_tensor("xp", [NU, P, 2, CHF], XDT, kind="ExternalInput").ap()
    wd = nc.dram_tensor("wd", [P, 2, K * COLS], XDT, kind="ExternalInput").ap()
    llog_d = nc.dram_tensor("llog", [P, NU], F32, kind="ExternalOutput").ap()
    nown_d = nc.dram_tensor("nown", [P, NU, 2 * TPB], F32, kind="ExternalOutput").ap()

    EXP = mybir.ActivationFunctionType.Exp
    LN = mybir.ActivationFunctionType.Ln

    with tile.TileContext(nc) as tc:
        with (
            tc.tile_pool(name="wpool", bufs=1) as wpool,
            # one slot per unit: loads never reuse a slot, so each DMA needs
            # no WAR/WAW wait (walrus allows only one sync-wait per DMA)
            tc.tile_pool(name="xpool", bufs=NU) as xpool,
            tc.tile_pool(name="ppool", bufs=3, space="PSUM") as ppool,
            tc.tile_pool(name="work", bufs=3) as work,
            tc.tile_pool(name="accs", bufs=1) as accs,
        ):
            wt = wpool.tile([P, 2, K * COLS], XDT)
            nc.sync.dma_start(wt[:], wd[:])
            llog_t = accs.tile([P, NU], F32)
            nown_t = accs.tile([P, NU, 2 * TPB], F32)

            for u, (t0, nb, tpb) in enumerate(units):
                g = nb * tpb
                ch = g * P
                xt = xpool.tile([P, 2, CHF], XDT, tag="xt")
                nc.sync.dma_start(xt[:, :, 0:ch], xp[u, :, :, 0:ch])
                ps = ppool.tile([P, 2, 512], F32, tag="ps")
                for t in range(g):
                    bk, ti = divmod(t, tpb)
                    kcls = int(tile_class[t0 + t])
                    c0 = ti * COLS
                    for c2 in range(2):
                        nc.tensor.matmul(
                            ps[:, bk, c0 : c0 + COLS],
                            xt[:, c2, t * P : (t + 1) * P],
                            wt[:, c2, kcls * COLS : (kcls + 1) * COLS],
                            start=(c2 == 0),
                            stop=(c2 == 1),
                        )
                psv = ps[:, 0:nb, 0 : tpb * COLS].rearrange(
                    "p b (t c) -> p b t c", c=COLS
                )
                neg = psv[:, :, :, M : M + K - 1]
                pos = psv[:, :, :, 0:M]
                own = psv[:, :, :, COLS - 1 : COLS]

                e = work.tile([P, 2, TPB, K - 1], F32, tag="e")
                ev = e[:, 0:nb, 0:tpb, :]
                nc.scalar.activation(ev, neg, EXP)
                sne = work.tile([P, 2, TPB], F32, tag="sne")
                snev = sne[:, 0:nb, 0:tpb]
                nc.vector.reduce_sum(snev, ev, axis=mybir.AxisListType.X)

                tb = work.tile([P, 2, TPB, M], F32, tag="tb")
                tbv = tb[:, 0:nb, 0:tpb, :]
                nc.scalar.activation(tbv, pos, EXP)
                nc.vector.tensor_add(
                    tbv, tbv, snev.unsqueeze(3).broadcast_to([P, nb, tpb, M])
                )
                nc.scalar.activation(tbv, tbv, LN, accum_out=llog_t[:, u : u + 1])

                nown_v = (
                    nown_t[:, u, 0:g].rearrange("p (b t) -> p b t", b=nb).unsqueeze(3)
                )
                nc.vector.tensor_copy(nown_v, own)

            nc.sync.dma_start(llog_d[:], llog_t[:])
            for u, (t0, nb, tpb) in enumerate(units):
                g = nb * tpb
                nc.sync.dma_start(nown_d[:, u, 0:g], nown_t[:, u, 0:g])
    _legalize_waits(nc)
    return nc


def prepare(feat, mask, bank):
    """Host-side: plan, per-core sharded inputs, weight matrix, pad count."""
    feat = np.ascontiguousarray(np.asarray(feat, dtype=np.float32))
    mask_flat = np.asarray(mask).reshape(-1).astype(np.int64)
    bank = np.asarray(bank, dtype=np.float32)

    idx_by_class, caps, seg, tile_class, units, T = _plan(mask_flat)
    NPX = T * P
    NU = len(units)
    CHF = 2 * TPB * P

    # [C, N] with the reference's pixel order n = (b*H + h)*W + w, staged as
    # [P, 2, NPX], then re-chunked unit-major [NU, P, 2, CHF] so each unit's
    # HWDGE load reads one contiguous 6KB run per partition row.
    f3 = feat.transpose(1, 0, 2, 3).reshape(2, P, NPIX)
    xs = []
    for c in range(NCORES):
        flat = np.zeros((P, 2, NPX), _np_xdt)
        for k in range(K):
            ix = idx_by_class[k][c::NCORES]
            s = int(seg[k])
            flat[:, :, s : s + len(ix)] = (
                f3[:, :, ix].transpose(1, 0, 2).astype(_np_xdt)
            )
        xc = np.zeros((NU, P, 2, CHF), _np_xdt)
        for u, (t0, nb, tpb) in enumerate(units):
            ch = nb * tpb * P
            xc[u, :, :, 0:ch] = flat[:, :, t0 * P : t0 * P + ch]
        xs.append(xc)
    n_pad_total = NCORES * NPX - NPIX

    bmean = bank.mean(axis=1)  # [K, C]
    wfull = np.zeros((C, K * COLS), np.float32)
    for k in range(K):
        wfull[:, k * COLS : k * COLS + M] = bank[k].T
        others = np.concatenate([np.arange(k), np.arange(k + 1, K)])
        wfull[:, k * COLS + M : k * COLS + M + K - 1] = bmean[others].T
        wfull[:, k * COLS + COLS - 1] = bmean[k]
    wfull /= TEMP
    wdat = np.ascontiguousarray(
        wfull.reshape(2, P, K * COLS).transpose(1, 0, 2).astype(_np_xdt)
    )

    return xs, wdat, tile_class, units, T, n_pad_total


def finish(results, n_pad_total, units):
    """Reduce per-core partial sums to the scalar loss (float64 on host)."""
    total = 0.0
    for r in results:
        total += r["llog"].sum(dtype=np.float64) / M
        for u, (t0, nb, tpb) in enumerate(units):
            total -= r["nown"][:, u, 0 : nb * tpb].sum(dtype=np.float64)
    total -= n_pad_total * np.log(19.0)
    return np.float32(total / NPIX)


def get_program(feat, mask, bank):
    xs, wdat, tile_class, units, T, n_pad_total = prepare(feat, mask, bank)
    key = (T, tuple(tile_class.tolist()))
    if key not in _prog_cache:
        _prog_cache[key] = _build(T, tile_class, units)
    return _prog_cache[key], xs, wdat, n_pad_total, units


def kernel(feat=None, mask=None, bank=None, _trace=False):
    nc, xs, wdat, n_pad_total, units = get_program(feat, mask, bank)
    in_maps = [{"xp": xs[c], "wd": wdat} for c in range(NCORES)]
    res = run_bass_kernel_spmd(
        nc, in_maps, core_ids=list(range(NCORES)), trace=_trace
    )
    loss = finish(res.results, n_pad_total, units)
    if _trace:
        return loss, res
    return loss
